# revision 13
# baseline (speedup 1.0000x reference)
"""Trainium2 Bass kernel for nn_CortexNetwork (dense_cnn, memory-bound).

Reference computation:
    patches[c,i,j,u,v] = x[c, rx[i]+u, ry[j]+v]
    aff[i,j] = sum_{c,u,v} patches * Wa
    exc[i,j] = sum_c prev[c,i,j] * sum_{x,y} We[c,i,j,x,y]   (inh likewise, Wi)
    out      = broadcast_c(relu(aff + 0.9*exc - 0.9*inh))

Strategy: tensor-parallel over the 36x36=1296 grid units, 162 units per
core on 8 cores (padded to 168 = 21 groups of 8 so every tile is a full
128 partitions = 16 channels x 8 units); every reduction is unit-local
so there are no collectives.  The kernel is HBM-bandwidth-bound, so all
streamed data is fp8_e4m3, quantized on the host with sum-preserving
rounding:

  * lateral We|-Wi rows (x64 scale) use error-diffusion rounding along
    each row, so the device's row sums match the f32 sums to ~1e-3;
  * afferent weights (x64) are rounded with the running product-sum
    carried against the fp8 patches (x16), GPTQ-style, so the device's
    dot products track the f32 products; patches are plain RTN fp8.

Measured end-to-end max-rel-error on the reference inputs is ~1.8e-3
(vs the 2e-2 gate) at 1/4 the f32 bytes.  Each (c,unit) row is a
3744-byte record: wa 576 | patch 576 | lateral 2592.  Mid-stream PE
records are shipped two groups per DMA (958 KB) for best HBM
efficiency; the 5 ScalarE records and the last two groups go singly
(the final group's afferent and lateral are separate DMAs so its
multiply can start early).

The 2592-wide lateral row sums are split across three engines so no
engine exceeds the ~30us DMA stream time.  16 groups go to the tensor
engine: their lateral columns are host-transposed into 128x128
LDWEIGHTS blocks consumed as chained matmuls against a constant fp8
ones vector, accumulating each group's per-(c,unit) row sum into one
PSUM column (the 32-wide tail is reduced on VectorE for most groups;
the final group instead carries it as a 21st transposed K=32 block so
nothing but matmuls gate its column).  The other 5 groups (placed
early) go to ScalarE as activation(Copy, scale=0.9*prev/64, accum_out).
VectorE runs the fused afferent multiply-reduce (scalar_tensor_tensor)
per group and the final relu.  The 16-channel sums are 0/1-selector
matmuls on PE (the afferent selector carries the 1/1024 dequant scale), emitted inline in
three column blocks ordered by data readiness -- the last group owns
output column 20 alone, and all other columns are relu'd and written
out while its data is still in flight.
"""

import numpy as np
import ml_dtypes

import concourse.bass as bass
import concourse.bacc as bacc
import concourse.mybir as mybir
from concourse import tile
from concourse.bass_utils import run_bass_kernel_spmd

N_CORES = 8
C = 16
GX = GY = 36
RF = 24
IMG = 64
GAMMA = 0.9

UNITS = GX * GY                  # 1296
PER_CORE = UNITS // N_CORES      # 162
S = 8                            # units per group (partition dim C*S=128)
T = 21                           # groups per core (168 units, 6 padded)
PADU = T * S                     # 168
FW = GX * GY                     # lateral free size per channel: 1296
FA = RF * RF                     # afferent free size per channel: 576
LCOL = 2 * FW                    # 2592 lateral elems per (c,unit)
KC = 20                          # full 128-chunks of the lateral dim on PE
LREM = LCOL - KC * 128           # 32-wide tail
AOFF = 0                         # record: wa | patch | lateral
POFF = FA
LOFF = 2 * FA                    # 1152
ROFF = LOFF + KC * 128           # 3712 row-major tail offset (non-final PE)
COLS = LOFF + LCOL               # 3744
COLS_L = LOFF + (KC + 1) * 128   # 3840: final group, tail as 21st block
WSCALE = 64.0                    # fp8 scale for We/Wi/Wa
PSCALE = 16.0                    # fp8 scale for patches
ACT_SET = (0, 2, 4, 6, 8)        # early groups whose lateral runs on ScalarE
PE_GROUPS = tuple(t for t in range(T) if t not in ACT_SET)
NPE = len(PE_GROUPS)             # 16
LAST = PE_GROUPS[-1]             # 20
# mid-stream PE groups shipped two per DMA; group 19 single, 20 split
PE_PAIRS = ((1, 3), (5, 7), (9, 10), (11, 12), (13, 14), (15, 16), (17, 18))
# sync-FIFO issue order of DMA units (pairs interleaved with ACT singles)
DMA_ORDER = (
    (0,), PE_PAIRS[0], (2,), PE_PAIRS[1], (4,), PE_PAIRS[2], (6,),
    PE_PAIRS[3], (8,), PE_PAIRS[4], PE_PAIRS[5], PE_PAIRS[6], (19,), (20,),
)
# output columns: PE groups except the last at 0..14, ACT at 15..19, and
# the last group alone at 20 so only that column waits on the final DMA
COL_OF = {t: i for i, t in enumerate(PE_GROUPS[:-1])}
COL_OF.update({t: NPE - 1 + j for j, t in enumerate(ACT_SET)})
COL_OF[LAST] = T - 1
PSCOL_OF = {t: i for i, t in enumerate(PE_GROUPS)}   # pslat/rrem column

F8 = ml_dtypes.float8_e4m3

_PROGRAM_CACHE = {}


def _build_program():
    f32 = mybir.dt.float32
    f8 = mybir.dt.float8e4
    bf16 = mybir.dt.bfloat16
    AL = mybir.AluOpType
    AF = mybir.ActivationFunctionType
    AX = mybir.AxisListType

    nc = bacc.Bacc(
        "TRN2", target_bir_lowering=False, debug=False, num_devices=N_CORES
    )
    biga_d = nc.dram_tensor("biga", [len(ACT_SET), 128, COLS], f8,
                            kind="ExternalInput").ap()
    bigp_d = nc.dram_tensor("bigp", [len(PE_PAIRS), 128, 2 * COLS], f8,
                            kind="ExternalInput").ap()
    bigs_d = nc.dram_tensor("bigs", [128, COLS], f8, kind="ExternalInput").ap()
    bigl_d = nc.dram_tensor("bigl", [128, COLS_L], f8, kind="ExternalInput").ap()
    possb_d = nc.dram_tensor("possb", [128, T], f32, kind="ExternalInput").ap()
    sel_d = nc.dram_tensor("sel", [128, S], f32, kind="ExternalInput").ap()
    sela_d = nc.dram_tensor("sela", [128, S], f32, kind="ExternalInput").ap()
    out_d = nc.dram_tensor("out", [S, T], f32, kind="ExternalOutput").ap()

    with tile.TileContext(nc) as tc:
        with (
            tc.tile_pool(name="wpair", bufs=4) as wpp,
            tc.tile_pool(name="wsing", bufs=3) as wsp,
            tc.tile_pool(name="cst", bufs=1) as cp,
            tc.tile_pool(name="junk", bufs=3) as jp,
            tc.tile_pool(name="fin", bufs=1) as fp,
            tc.tile_pool(name="ps", bufs=1, space="PSUM") as pp,
        ):
            possb = cp.tile([128, T], f32, tag="possb")
            sel = cp.tile([128, S], f32, tag="sel")
            sela = cp.tile([128, S], f32, tag="sela")
            ones = cp.tile([128, 1], f8, tag="ones")
            plat = cp.tile([128, T], f32, tag="plat")
            paff = cp.tile([128, T], f32, tag="paff")
            rrem = cp.tile([128, NPE - 1], f32, tag="rrem")
            nc.gpsimd.dma_start(possb[:], possb_d[:])
            nc.gpsimd.dma_start(sel[:], sel_d[:])
            nc.gpsimd.dma_start(sela[:], sela_d[:])
            nc.vector.memset(ones[:], 1.0)

            pslat = pp.tile([128, NPE], f32, tag="pslat")
            psum = pp.tile([S, T], f32, tag="ps")
            res = fp.tile([S, T], f32, tag="res")

            def chansum(c0, c1):
                nc.tensor.matmul(psum[:, c0:c1], sel[:], plat[:, c0:c1],
                                 start=True, stop=False)
                nc.tensor.matmul(psum[:, c0:c1], sela[:], paff[:, c0:c1],
                                 start=False, stop=True)

            def group_ops(t, w, off):
                col = COL_OF[t]
                if t in ACT_SET:
                    j = jp.tile([128, LCOL], f32, tag="jlat")
                    nc.scalar.activation(
                        j[:], w[:, off + LOFF:off + COLS], AF.Copy,
                        scale=possb[:, col:col + 1],
                        accum_out=plat[:, col:col + 1],
                    )
                else:
                    pcol = PSCOL_OF[t]
                    nk = KC + 1 if t == LAST else KC
                    for k in range(nk):
                        o = off + LOFF + 128 * k
                        kp = 128 if k < KC else LREM
                        nc.tensor.matmul(
                            pslat[:, pcol:pcol + 1],
                            w[0:kp, o:o + 128], ones[0:kp, :],
                            start=(k == 0), stop=(k == nk - 1),
                        )
                    if t != LAST:
                        nc.vector.tensor_reduce(
                            rrem[:, pcol:pcol + 1], w[:, off + ROFF:off + COLS],
                            axis=AX.X, op=AL.add,
                        )
                ja = jp.tile([128, FA], bf16, tag="jaff")
                nc.vector.scalar_tensor_tensor(
                    ja[:], w[:, off + AOFF:off + AOFF + FA], 1.0,
                    w[:, off + POFF:off + POFF + FA],
                    op0=AL.mult, op1=AL.mult,
                    accum_out=paff[:, col:col + 1],
                )

            na = 0
            np_ = 0
            for unit in DMA_ORDER:
                t0 = unit[0]
                if t0 == LAST:
                    w = wsp.tile([128, COLS_L], f8, tag="wlast")
                    nc.sync.dma_start(w[:, 0:LOFF], bigl_d[:, 0:LOFF])
                    nc.sync.dma_start(w[:, LOFF:COLS_L], bigl_d[:, LOFF:COLS_L])
                    group_ops(t0, w, 0)
                elif len(unit) == 2:
                    w = wpp.tile([128, 2 * COLS], f8, tag="wpair")
                    nc.sync.dma_start(w[:], bigp_d[np_])
                    np_ += 1
                    group_ops(unit[0], w, 0)
                    group_ops(unit[1], w, COLS)
                elif t0 in ACT_SET:
                    w = wsp.tile([128, COLS], f8, tag="wsing")
                    nc.sync.dma_start(w[:], biga_d[na])
                    na += 1
                    group_ops(t0, w, 0)
                else:                                    # group 19 single
                    w = wsp.tile([128, COLS], f8, tag="wsing")
                    nc.sync.dma_start(w[:], bigs_d[:])
                    group_ops(t0, w, 0)
                # early blocks, nudged ahead in the scheduler's heap
                if t0 == ACT_SET[-1]:
                    with tc.high_priority():
                        chansum(NPE - 1, T - 1)          # ACT cols 15..19
                elif unit == (19,):
                    with tc.high_priority():
                        # PE cols 0..14: merge remainder+possb, sum, write out
                        nc.vector.tensor_add(plat[:, 0:NPE - 1],
                                             pslat[:, 0:NPE - 1], rrem[:])
                        nc.vector.tensor_mul(plat[:, 0:NPE - 1],
                                             plat[:, 0:NPE - 1],
                                             possb[:, 0:NPE - 1])
                        chansum(0, NPE - 1)
                        nc.vector.tensor_scalar_max(res[:, 0:T - 1],
                                                    psum[:, 0:T - 1], 0.0)
                        # scalar-engine HWDGE ring: must not sit ahead of
                        # the last input DMAs in the sync FIFO
                        nc.scalar.dma_start(out_d[:, 0:T - 1], res[:, 0:T - 1])

            # final column: nothing but the last group's matmuls gate it
            nc.vector.tensor_mul(plat[:, T - 1:T], pslat[:, NPE - 1:NPE],
                                 possb[:, T - 1:T])
            chansum(T - 1, T)
            nc.vector.tensor_scalar_max(res[:, T - 1:T], psum[:, T - 1:T], 0.0)
            nc.sync.dma_start(out_d[:, T - 1:T], res[:, T - 1:T])

    nc.compile()
    return nc


def _get_program():
    if "nc" not in _PROGRAM_CACHE:
        _PROGRAM_CACHE["nc"] = _build_program()
    return _PROGRAM_CACHE["nc"]


def _f8(v):
    return np.clip(v, -240.0, 240.0).astype(F8)


def _ed_rows(w, chunk):
    """fp8 quantize along the last axis with error-diffusion so each
    chunk's sum is preserved to ~one fp8 step."""
    r, n = w.shape
    wv = w.reshape(r * (n // chunk), chunk)
    q = np.empty(wv.shape, F8)
    carry = np.zeros(wv.shape[0], np.float32)
    for k in range(chunk):
        t = wv[:, k] + carry
        qk = _f8(t)
        q[:, k] = qk
        carry = t - qk.astype(np.float32)
    return q.reshape(r, n)


def _gptq_wa(wa_s, pq, t_s):
    """fp8-round scaled afferent weights with the running product-sum
    carried against the fp8 patches, so sum(q*pq) tracks sum(t_s)."""
    r, n = wa_s.shape
    pqf = pq.astype(np.float32)
    q = np.empty((r, n), F8)
    carry = np.zeros(r, np.float32)
    for k in range(n):
        tk = t_s[:, k] + carry
        pk = pqf[:, k]
        safe = np.where(pk == 0, 1.0, pk)
        v = np.where(pk != 0, tk / safe, wa_s[:, k])
        qk = _f8(v)
        q[:, k] = qk
        carry = tk - qk.astype(np.float32) * pk
    return q


def _transpose_lat(lt, nchunk):
    """[128 cu, nchunk*128 lat] -> LDWEIGHTS blocks: out[p, 128k+f] =
    lt[f, 128k+p]."""
    n = nchunk * 128
    return lt[:, 0:n].reshape(128, nchunk, 128).transpose(2, 1, 0).reshape(128, n)


def _prep_in_maps(inputs):
    x = np.asarray(inputs["x"], dtype=np.float32)
    prev = np.asarray(inputs["prev_activity"], dtype=np.float32).reshape(C, UNITS)
    wa = np.asarray(inputs["afferent_weights"], dtype=np.float32).reshape(C, UNITS, FA)
    we = np.asarray(inputs["ex_lateral_weights"], dtype=np.float32).reshape(C, UNITS, FW)
    wi = np.asarray(inputs["in_lateral_weights"], dtype=np.float32).reshape(C, UNITS, FW)
    rx = np.asarray(inputs["rx"]).astype(np.int64)
    ry = np.asarray(inputs["ry"]).astype(np.int64)

    u = np.arange(RF)
    ix = rx[:, None] + u                     # [GX, RF]
    iy = ry[:, None] + u                     # [GY, RF]
    px = x[:, ix, :]                         # [C, GX, RF, IMG]
    patches = px[:, :, :, iy]                # [C, GX, RF, GY, RF]
    patches = np.ascontiguousarray(patches.transpose(0, 1, 3, 2, 4))
    patches = patches.reshape(C * UNITS, FA)

    lat = np.concatenate([we, -wi], axis=2).reshape(C * UNITS, LCOL)
    lat_q = _ed_rows(lat * WSCALE, 324)                       # [C*U, 2592] f8
    pq = _f8(patches * PSCALE)                                # [C*U, 576] f8
    wa2 = wa.reshape(C * UNITS, FA)
    t_s = (wa2 * patches) * (WSCALE * PSCALE)
    wa_q = _gptq_wa(wa2 * WSCALE, pq, t_s)                    # [C*U, 576] f8

    lat_q = lat_q.reshape(C, UNITS, LCOL)
    affcat = np.concatenate(
        [wa_q.reshape(C, UNITS, FA), pq.reshape(C, UNITS, FA)], axis=2
    )                                                          # [C, U, 1152]
    prevf = prev * (GAMMA / WSCALE)

    sel = (np.arange(128)[:, None] % S == np.arange(S)[None, :]).astype(np.float32)
    sela = sel * np.float32(1.0 / (WSCALE * PSCALE))

    in_maps = []
    for kcore in range(N_CORES):
        n0 = kcore * PER_CORE
        lq = np.zeros((C, PADU, LCOL), F8)
        lq[:, :PER_CORE] = lat_q[:, n0:n0 + PER_CORE]
        af = np.zeros((C, PADU, 2 * FA), F8)
        af[:, :PER_CORE] = affcat[:, n0:n0 + PER_CORE]
        # partition-major [T, 128, .] with row p = c*S + s
        lqg = lq.reshape(C, T, S, LCOL).transpose(1, 0, 2, 3).reshape(T, 128, LCOL)
        afg = af.reshape(C, T, S, 2 * FA).transpose(1, 0, 2, 3).reshape(T, 128, 2 * FA)

        def record(t):
            r = np.empty((128, COLS), F8)
            r[:, 0:LOFF] = afg[t]
            if t in ACT_SET:
                r[:, LOFF:COLS] = lqg[t]
            else:
                r[:, LOFF:ROFF] = _transpose_lat(lqg[t], KC)
                r[:, ROFF:COLS] = lqg[t][:, KC * 128:LCOL]
            return r

        biga = np.stack([record(t) for t in ACT_SET])
        bigp = np.stack([
            np.concatenate([record(a), record(b)], axis=1) for a, b in PE_PAIRS
        ])
        bigs = record(19)
        bigl = np.zeros((128, COLS_L), F8)
        bigl[:, 0:LOFF] = afg[LAST]
        bigl[:, LOFF:ROFF] = _transpose_lat(lqg[LAST], KC)
        # 21st K=32 block: [p, ROFF+f] = lat[f, 2560+p] for p < 32
        bigl[0:LREM, ROFF:ROFF + 128] = lqg[LAST][:, KC * 128:LCOL].T

        pv = np.zeros((C, PADU), np.float32)
        pv[:, :PER_CORE] = prevf[:, n0:n0 + PER_CORE]
        pv = pv.reshape(C, T, S).transpose(0, 2, 1).reshape(128, T)
        pvp = np.empty_like(pv)
        for t in range(T):
            pvp[:, COL_OF[t]] = pv[:, t]
        in_maps.append({
            "biga": np.ascontiguousarray(biga),
            "bigp": np.ascontiguousarray(bigp),
            "bigs": np.ascontiguousarray(bigs),
            "bigl": np.ascontiguousarray(bigl),
            "possb": np.ascontiguousarray(pvp),
            "sel": sel,
            "sela": sela,
        })
    return in_maps


def _assemble_output(results):
    act = np.empty(UNITS, np.float32)
    for kcore in range(N_CORES):
        o = np.asarray(results[kcore]["out"])            # [S, T] permuted cols
        for t in range(T):
            n0 = kcore * PER_CORE + t * S
            lim = (kcore + 1) * PER_CORE
            if n0 >= lim:
                break
            nn = min(S, lim - n0)
            act[n0:n0 + nn] = o[:nn, COL_OF[t]]
    out = np.broadcast_to(act.reshape(1, GX, GY), (C, GX, GY))
    return np.ascontiguousarray(out, dtype=np.float32)


def kernel(**inputs):
    nc = _get_program()
    in_maps = _prep_in_maps(inputs)
    res = run_bass_kernel_spmd(nc, in_maps, core_ids=list(range(N_CORES)))
    return _assemble_output(res.results)


# revision 14
# speedup vs baseline: 1.0801x; 1.0801x over previous
"""Trainium2 Bass kernel for nn_CortexNetwork (dense_cnn, memory-bound).

Reference computation:
    patches[c,i,j,u,v] = x[c, rx[i]+u, ry[j]+v]
    aff[i,j] = sum_{c,u,v} patches * Wa
    exc[i,j] = sum_c prev[c,i,j] * sum_{x,y} We[c,i,j,x,y]   (inh likewise, Wi)
    out      = broadcast_c(relu(aff + 0.9*exc - 0.9*inh))

Strategy: tensor-parallel over the 36x36=1296 grid units, 162 units per
core on 8 cores (padded to 168 = 21 groups of 8 so every tile is a full
128 partitions = 16 channels x 8 units); every reduction is unit-local
so there are no collectives.  The kernel is HBM-bandwidth-bound, so all
streamed data is fp8_e4m3, quantized on the host with sum-preserving
rounding:

  * lateral We|-Wi rows (x64 scale) use error-diffusion rounding along
    each row, so the device's row sums match the f32 sums to ~1e-3;
  * afferent weights (x64) are rounded with the running product-sum
    carried against the fp8 patches (x16), GPTQ-style, so the device's
    dot products track the f32 products; patches are plain RTN fp8.

Measured end-to-end max-rel-error on the reference inputs is ~1.8e-3
(vs the 2e-2 gate) at 1/4 the f32 bytes.

The 2592-wide lateral row sums are split across three engines so no
engine exceeds the ~30us DMA stream time.  16 groups go to the tensor
engine: their lateral columns are host-transposed (zero-padded to
21x128) into 128x128 LDWEIGHTS blocks consumed as chained matmuls
against a constant fp8 ones vector, accumulating each group's
per-(c,unit) row sum into one PSUM column.  5 groups go to ScalarE as
activation(Copy, scale=0.9*prev/64, accum_out).  VectorE runs the fused
afferent multiply-reduce (scalar_tensor_tensor) per group, the
0.9*prev/64 multiply for the PE block, and the final relu.  The
16-channel sums are 0/1-selector matmuls on PE (the afferent selector
carries the 1/1024 dequant scale).

Tail packing: the last-streamed group owns output column 20 alone and
its record is sent as two DMAs (afferent first); every other column's
channel sums, relu and output DMA (on the scalar-engine HWDGE ring, so
the sync FIFO stays clear) are emitted right after their dependencies
mid-loop, so after the final DMA lands only one group's matmuls, one
merge, one 1-column channel sum and a 32-byte output write remain.
"""

import numpy as np
import ml_dtypes

import concourse.bass as bass
import concourse.bacc as bacc
import concourse.mybir as mybir
from concourse import tile
from concourse.bass_utils import run_bass_kernel_spmd

N_CORES = 8
C = 16
GX = GY = 36
RF = 24
IMG = 64
GAMMA = 0.9

UNITS = GX * GY                  # 1296
PER_CORE = UNITS // N_CORES      # 162
S = 8                            # units per group (partition dim C*S=128)
T = 21                           # groups per core (168 units, 6 padded)
PADU = T * S                     # 168
FW = GX * GY                     # lateral free size per channel: 1296
FA = RF * RF                     # afferent free size per channel: 576
LCOL = 2 * FW                    # 2592 lateral elems per (c,unit)
KC = 21                          # 128-chunks of the padded lateral dim
LPAD = KC * 128                  # 2688
COLS_A = LCOL + 2 * FA           # 3744  (ScalarE-group record)
COLS_P = LPAD + 2 * FA           # 3840  (PE-group record, lateral transposed)
WSCALE = 64.0                    # fp8 scale for We/Wi/Wa
PSCALE = 16.0                    # fp8 scale for patches
ACT_SET = (2, 6, 10, 14, 18)     # groups whose lateral runs on ScalarE
PE_GROUPS = tuple(t for t in range(T) if t not in ACT_SET)
NPE = len(PE_GROUPS)             # 16
LAST = PE_GROUPS[-1]             # 20
# output columns: PE groups except the last at 0..14, ACT at 15..19, and
# the last group alone at 20 so only that column waits on the final DMA
COL_OF = {t: i for i, t in enumerate(PE_GROUPS[:-1])}
COL_OF.update({t: NPE - 1 + j for j, t in enumerate(ACT_SET)})
COL_OF[LAST] = T - 1
PSCOL_OF = {t: i for i, t in enumerate(PE_GROUPS)}   # pslat column

F8 = ml_dtypes.float8_e4m3

_PROGRAM_CACHE = {}


def _build_program():
    f32 = mybir.dt.float32
    f8 = mybir.dt.float8e4
    bf16 = mybir.dt.bfloat16
    AL = mybir.AluOpType
    AF = mybir.ActivationFunctionType

    nc = bacc.Bacc(
        "TRN2", target_bir_lowering=False, debug=False, num_devices=N_CORES
    )
    bigp_d = nc.dram_tensor("bigp", [NPE, 128, COLS_P], f8, kind="ExternalInput").ap()
    biga_d = nc.dram_tensor("biga", [T - NPE, 128, COLS_A], f8, kind="ExternalInput").ap()
    possb_d = nc.dram_tensor("possb", [128, T], f32, kind="ExternalInput").ap()
    sel_d = nc.dram_tensor("sel", [128, S], f32, kind="ExternalInput").ap()
    sela_d = nc.dram_tensor("sela", [128, S], f32, kind="ExternalInput").ap()
    out_d = nc.dram_tensor("out", [S, T], f32, kind="ExternalOutput").ap()

    with tile.TileContext(nc) as tc:
        with (
            tc.tile_pool(name="wp", bufs=6) as wpp,
            tc.tile_pool(name="wa", bufs=3) as wap,
            tc.tile_pool(name="cst", bufs=1) as cp,
            tc.tile_pool(name="junk", bufs=3) as jp,
            tc.tile_pool(name="fin", bufs=1) as fp,
            tc.tile_pool(name="ps", bufs=1, space="PSUM") as pp,
        ):
            possb = cp.tile([128, T], f32, tag="possb")
            sel = cp.tile([128, S], f32, tag="sel")
            sela = cp.tile([128, S], f32, tag="sela")
            ones = cp.tile([128, 1], f8, tag="ones")
            plat = cp.tile([128, T], f32, tag="plat")
            paff = cp.tile([128, T], f32, tag="paff")
            nc.gpsimd.dma_start(possb[:], possb_d[:])
            nc.gpsimd.dma_start(sel[:], sel_d[:])
            nc.gpsimd.dma_start(sela[:], sela_d[:])
            nc.vector.memset(ones[:], 1.0)

            pslat = pp.tile([128, NPE], f32, tag="pslat")
            psum = pp.tile([S, T], f32, tag="ps")
            res = fp.tile([S, T], f32, tag="res")

            def chansum(c0, c1):
                nc.tensor.matmul(psum[:, c0:c1], sel[:], plat[:, c0:c1],
                                 start=True, stop=False)
                nc.tensor.matmul(psum[:, c0:c1], sela[:], paff[:, c0:c1],
                                 start=False, stop=True)

            for t in range(T):
                col = COL_OF[t]
                if t in ACT_SET:
                    w = wap.tile([128, COLS_A], f8, tag="wa")
                    nc.sync.dma_start(w[:], biga_d[col - (NPE - 1)])
                    j = jp.tile([128, LCOL], f32, tag="jlat")
                    nc.scalar.activation(
                        j[:], w[:, 0:LCOL], AF.Copy,
                        scale=possb[:, col:col + 1],
                        accum_out=plat[:, col:col + 1],
                    )
                    aoff = LCOL
                else:
                    pcol = PSCOL_OF[t]
                    w = wpp.tile([128, COLS_P], f8, tag="wp")
                    if t == LAST:
                        nc.sync.dma_start(w[:, LPAD:COLS_P],
                                          bigp_d[pcol, :, LPAD:COLS_P])
                        nc.sync.dma_start(w[:, 0:LPAD], bigp_d[pcol, :, 0:LPAD])
                    else:
                        nc.sync.dma_start(w[:], bigp_d[pcol])
                    for k in range(KC):
                        nc.tensor.matmul(
                            pslat[:, pcol:pcol + 1],
                            w[:, 128 * k:128 * (k + 1)], ones[:],
                            start=(k == 0), stop=(k == KC - 1),
                        )
                    aoff = LPAD
                ja = jp.tile([128, FA], bf16, tag="jaff")
                nc.vector.scalar_tensor_tensor(
                    ja[:], w[:, aoff:aoff + FA], 1.0, w[:, aoff + FA:aoff + 2 * FA],
                    op0=AL.mult, op1=AL.mult,
                    accum_out=paff[:, col:col + 1],
                )
                # early blocks (emission order = scheduler priority)
                if t == ACT_SET[-1]:
                    chansum(NPE - 1, T - 1)              # ACT cols 15..19
                elif t == PE_GROUPS[-2]:
                    # PE cols 0..14: apply 0.9*prev/64, sum, relu, write out
                    nc.vector.tensor_mul(plat[:, 0:NPE - 1],
                                         pslat[:, 0:NPE - 1],
                                         possb[:, 0:NPE - 1])
                    chansum(0, NPE - 1)
                    nc.vector.tensor_scalar_max(res[:, 0:T - 1],
                                                psum[:, 0:T - 1], 0.0)
                    # scalar-engine HWDGE ring keeps the sync FIFO clear
                    nc.scalar.dma_start(out_d[:, 0:T - 1], res[:, 0:T - 1])

            # final column: only the last group's data gates this
            nc.vector.tensor_mul(plat[:, T - 1:T], pslat[:, NPE - 1:NPE],
                                 possb[:, T - 1:T])
            chansum(T - 1, T)
            nc.vector.tensor_scalar_max(res[:, T - 1:T], psum[:, T - 1:T], 0.0)
            nc.sync.dma_start(out_d[:, T - 1:T], res[:, T - 1:T])

    nc.compile()
    return nc


def _get_program():
    if "nc" not in _PROGRAM_CACHE:
        _PROGRAM_CACHE["nc"] = _build_program()
    return _PROGRAM_CACHE["nc"]


def _f8(v):
    return np.clip(v, -240.0, 240.0).astype(F8)


def _ed_rows(w, chunk):
    """fp8 quantize along the last axis with error-diffusion so each
    chunk's sum is preserved to ~one fp8 step."""
    r, n = w.shape
    wv = w.reshape(r * (n // chunk), chunk)
    q = np.empty(wv.shape, F8)
    carry = np.zeros(wv.shape[0], np.float32)
    for k in range(chunk):
        t = wv[:, k] + carry
        qk = _f8(t)
        q[:, k] = qk
        carry = t - qk.astype(np.float32)
    return q.reshape(r, n)


def _gptq_wa(wa_s, pq, t_s):
    """fp8-round scaled afferent weights with the running product-sum
    carried against the fp8 patches, so sum(q*pq) tracks sum(t_s)."""
    r, n = wa_s.shape
    pqf = pq.astype(np.float32)
    q = np.empty((r, n), F8)
    carry = np.zeros(r, np.float32)
    for k in range(n):
        tk = t_s[:, k] + carry
        pk = pqf[:, k]
        safe = np.where(pk == 0, 1.0, pk)
        v = np.where(pk != 0, tk / safe, wa_s[:, k])
        qk = _f8(v)
        q[:, k] = qk
        carry = tk - qk.astype(np.float32) * pk
    return q


def _prep_in_maps(inputs):
    x = np.asarray(inputs["x"], dtype=np.float32)
    prev = np.asarray(inputs["prev_activity"], dtype=np.float32).reshape(C, UNITS)
    wa = np.asarray(inputs["afferent_weights"], dtype=np.float32).reshape(C, UNITS, FA)
    we = np.asarray(inputs["ex_lateral_weights"], dtype=np.float32).reshape(C, UNITS, FW)
    wi = np.asarray(inputs["in_lateral_weights"], dtype=np.float32).reshape(C, UNITS, FW)
    rx = np.asarray(inputs["rx"]).astype(np.int64)
    ry = np.asarray(inputs["ry"]).astype(np.int64)

    u = np.arange(RF)
    ix = rx[:, None] + u                     # [GX, RF]
    iy = ry[:, None] + u                     # [GY, RF]
    px = x[:, ix, :]                         # [C, GX, RF, IMG]
    patches = px[:, :, :, iy]                # [C, GX, RF, GY, RF]
    patches = np.ascontiguousarray(patches.transpose(0, 1, 3, 2, 4))
    patches = patches.reshape(C * UNITS, FA)

    lat = np.concatenate([we, -wi], axis=2).reshape(C * UNITS, LCOL)
    lat_q = _ed_rows(lat * WSCALE, 324)                       # [C*U, 2592] f8
    pq = _f8(patches * PSCALE)                                # [C*U, 576] f8
    wa2 = wa.reshape(C * UNITS, FA)
    t_s = (wa2 * patches) * (WSCALE * PSCALE)
    wa_q = _gptq_wa(wa2 * WSCALE, pq, t_s)                    # [C*U, 576] f8

    lat_q = lat_q.reshape(C, UNITS, LCOL)
    affcat = np.concatenate(
        [wa_q.reshape(C, UNITS, FA), pq.reshape(C, UNITS, FA)], axis=2
    )                                                          # [C, U, 1152]
    prevf = prev * (GAMMA / WSCALE)

    sel = (np.arange(128)[:, None] % S == np.arange(S)[None, :]).astype(np.float32)
    sela = sel * np.float32(1.0 / (WSCALE * PSCALE))

    in_maps = []
    for kcore in range(N_CORES):
        n0 = kcore * PER_CORE
        lq = np.zeros((C, PADU, LCOL), F8)
        lq[:, :PER_CORE] = lat_q[:, n0:n0 + PER_CORE]
        af = np.zeros((C, PADU, 2 * FA), F8)
        af[:, :PER_CORE] = affcat[:, n0:n0 + PER_CORE]
        # partition-major [T, 128, .] with row p = c*S + s
        lqg = lq.reshape(C, T, S, LCOL).transpose(1, 0, 2, 3).reshape(T, 128, LCOL)
        afg = af.reshape(C, T, S, 2 * FA).transpose(1, 0, 2, 3).reshape(T, 128, 2 * FA)

        bigp = np.zeros((NPE, 128, COLS_P), F8)
        biga = np.zeros((T - NPE, 128, COLS_A), F8)
        for t in range(T):
            if t in ACT_SET:
                ai = COL_OF[t] - (NPE - 1)
                biga[ai, :, 0:LCOL] = lqg[t]
                biga[ai, :, LCOL:COLS_A] = afg[t]
            else:
                # transpose lateral: tile[p, 128k+f] = lat[f, 128k+p], 0-pad
                lt = np.zeros((128, LPAD), F8)
                lt[:, 0:LCOL] = lqg[t]
                pi = PSCOL_OF[t]
                bigp[pi, :, 0:LPAD] = (
                    lt.reshape(128, KC, 128).transpose(2, 1, 0).reshape(128, LPAD)
                )
                bigp[pi, :, LPAD:COLS_P] = afg[t]

        pv = np.zeros((C, PADU), np.float32)
        pv[:, :PER_CORE] = prevf[:, n0:n0 + PER_CORE]
        pv = pv.reshape(C, T, S).transpose(0, 2, 1).reshape(128, T)
        pvp = np.empty_like(pv)
        for t in range(T):
            pvp[:, COL_OF[t]] = pv[:, t]
        in_maps.append({
            "bigp": np.ascontiguousarray(bigp),
            "biga": np.ascontiguousarray(biga),
            "possb": np.ascontiguousarray(pvp),
            "sel": sel,
            "sela": sela,
        })
    return in_maps


def _assemble_output(results):
    act = np.empty(UNITS, np.float32)
    for kcore in range(N_CORES):
        o = np.asarray(results[kcore]["out"])            # [S, T] permuted cols
        for t in range(T):
            n0 = kcore * PER_CORE + t * S
            lim = (kcore + 1) * PER_CORE
            if n0 >= lim:
                break
            nn = min(S, lim - n0)
            act[n0:n0 + nn] = o[:nn, COL_OF[t]]
    out = np.broadcast_to(act.reshape(1, GX, GY), (C, GX, GY))
    return np.ascontiguousarray(out, dtype=np.float32)


def kernel(**inputs):
    nc = _get_program()
    in_maps = _prep_in_maps(inputs)
    res = run_bass_kernel_spmd(nc, in_maps, core_ids=list(range(N_CORES)))
    return _assemble_output(res.results)
